# revision 1
# baseline (speedup 1.0000x reference)
"""DenseCRF mean-field inference on 8 Trainium2 NeuronCores.

Math: per image, 5 iterations of
    q_hat = U + 4*((q/n) @ K)/n + 2*(q @ S);  q = softmax(q_hat, axis=0)
with K[i,j] = exp(-0.5*d2(i,j)) the dense 9216x9216 bilateral kernel,
n = sqrt(K @ 1), S = kron(B,B) the separable (normalized, banded) 71x71
spatial kernel.

Sharding: K's columns over 8 cores; each core holds K[:, mine] =
[9216, 1152] bf16 (~21 MB) resident in SBUF. Per iteration each core
computes its 1152 columns of the update, softmaxes them pixel-major, and
an AllGather redistributes the full q.

Pipeline structure (driven by the TimelineSim cost model, 757us -> 495us):
  * Build: E=exp(T') via bf16 feature matmuls -- exact hi/lo split of the
    color features (products of bf16 pairs are exact in f32 PSUM, so d2
    carries no cancellation error) -- with ACT accum_out producing
    colsum(K) partials for free (rowsum of my block = colsum restricted
    to my j's, by K's symmetry). ACT-exp-bound at ~105us.
  * q0 = softmax(U) AllGathers at ~6us, fully hidden inside the build;
    the colsum partials AllGather in two halves (the first fires
    mid-build) so iteration 1's first 36 chunks start right at build end.
  * No M-rescale pass: the gathered q is scaled by rn_i in place (per-core
    slices as each gather DMA lands), rn_j folds into the pixel-major
    softmax input (scalar_tensor_tensor with a per-partition rn), and the
    +2S spatial term is computed separably each iteration -- stage-1
    matmul vs Bmy = 2*B[:,mine], 21 strided PE transposes, 3 class-batched
    stage-2 matmuls vs B, all interleaved into the matvec's PE stream,
    with a contiguous DRAM bounce landing sp pixel-major.
  * Iterations >= 2 run the matvec bank-major (3 PSUM column banks x 72
    chunks) so each bank's columns finalize early and the softmax of bank
    b-1 overlaps bank b's matmuls; per-bank reciprocal/scale/store lets
    the AllGather trigger right after the last bank's short chain.
  * rn extraction for the SPMD-uniform program: a one-hot selector matmul
    (per-core input) pulls the core's own 4*rn_j out of the transposed
    colsum pipeline (a program cannot slice by core id at compile time).
"""

import numpy as np
import ml_dtypes

H = 96
W = 96
P = H * W            # 9216 pixels
L = 21               # classes
NCORES = 8
PSH = P // NCORES    # 1152 pixels per core
NI = P // 128        # 72 contraction chunks
NJ = PSH // 128      # 9 output-pixel chunks per core
NYS = PSH // W       # 12 image rows per core
NSLAB = 8            # lhsT feature slabs of 1152 columns
NF = 15              # extended feature rows
NG = 3               # stage-2 class groups (7 classes each)
LG = L // NG
SXY_BF = 70.0
SC_BF = 12.0
LN4 = float(np.log(4.0))
QLEN = NJ * 128 * L      # 24192 bf16 per q shard
CLEN = 128 * NI          # 9216 colsum partials
XL = W * L               # 2016

_bf16 = ml_dtypes.bfloat16

_CACHE = {}
TRACE = False      # set by test harness for profiling runs
LAST_RESULT = None


# ----------------------------------------------------------------------------
# host-side input prep
# ----------------------------------------------------------------------------

def _host_constants():
    if "consts" in _CACHE:
        return _CACHE["consts"]
    # 1D spatial gaussian band matrix B[a,b] = exp(-(a-b)^2/72)/z, |a-b|<=35
    sig_sq = 36.0
    rr = 35
    g1 = np.exp(-((np.arange(2 * rr + 1, dtype=np.float64) - rr) ** 2)
                / (2 * sig_sq))
    z = g1.sum()
    idx = np.arange(H)
    d = idx[:, None] - idx[None, :]
    B = np.where(np.abs(d) <= rr,
                 np.exp(-(d.astype(np.float64) ** 2) / (2 * sig_sq)) / z, 0.0)
    ys = np.arange(P) // W
    xs = np.arange(P) % W
    _CACHE["consts"] = (B, ys, xs)
    return _CACHE["consts"]


def _feature_rows(ref):
    """15 extended-feature rows, PL [15,P] (lhsT side) / QR [15,P] (rhs)."""
    _, ys, xs = _host_constants()
    r2 = np.asarray(ref, np.float64).reshape(3, P)
    yf = (ys / SXY_BF).astype(_bf16)
    xf = (xs / SXY_BF).astype(_bf16)
    ch = (r2 / SC_BF).astype(_bf16)                       # [3, P]
    cl = (r2 / SC_BF - ch.astype(np.float64)).astype(_bf16)
    fy = yf.astype(np.float64)
    fx = xf.astype(np.float64)
    fc = ch.astype(np.float64) + cl.astype(np.float64)
    dvec = -0.5 * (fy ** 2 + fx ** 2 + (fc ** 2).sum(0))  # [P]
    dh = dvec.astype(_bf16)
    dl = (dvec - dh.astype(np.float64)).astype(_bf16)
    ones = np.ones(P, _bf16)
    PL = np.stack([yf, xf, ch[0], ch[1], ch[2], ch[0], ch[1], ch[2],
                   cl[0], cl[1], cl[2], dh, dl, ones, ones], 0)
    QR = np.stack([yf, xf, ch[0], ch[1], ch[2], cl[0], cl[1], cl[2],
                   ch[0], ch[1], ch[2], ones, ones, dh, dl], 0)
    return np.ascontiguousarray(PL), np.ascontiguousarray(QR)


# ----------------------------------------------------------------------------
# device program
# ----------------------------------------------------------------------------

def _build_bass(niters=5):
    key = ("nc", niters)
    if key in _CACHE:
        return _CACHE[key]

    import concourse.bass as bass
    import concourse.bacc as bacc
    import concourse.tile as tile
    import concourse.mybir as mybir
    from concourse.masks import make_identity

    f32 = mybir.dt.float32
    bf16 = mybir.dt.bfloat16
    AF = mybir.ActivationFunctionType
    ALU = mybir.AluOpType

    nc = bacc.Bacc("TRN2", num_devices=NCORES)

    unary_m = nc.dram_tensor("unary_m", [L, PSH], f32, kind="ExternalInput")
    plb = nc.dram_tensor("plb", [NF, P], bf16, kind="ExternalInput")
    qrb = nc.dram_tensor("qrb", [NF, PSH], bf16, kind="ExternalInput")
    bmat = nc.dram_tensor("bmat", [W, W], bf16, kind="ExternalInput")
    bmy = nc.dram_tensor("bmy", [H, NYS], bf16, kind="ExternalInput")
    selb = nc.dram_tensor("selb", [NI, NJ], bf16, kind="ExternalInput")
    qout = nc.dram_tensor("qout", [NJ, 128, L], f32, kind="ExternalOutput")

    rg = [list(range(NCORES))]
    COLS = ((0, 512), (512, 512), (1024, 128))
    HXL = XL // 2
    NIH = NI // 2          # 36: partials-AG half
    CLH = CLEN // 2

    with tile.TileContext(nc) as tc:
        with tc.tile_pool(name="dram", bufs=1, space="DRAM") as dram:
            qsh_d = dram.tile([NJ, 128, L], bf16)    # my q shard, AG input
            qfl_d = dram.tile([NCORES, QLEN], bf16)  # AG output
            pin_d = dram.tile([2, CLH], bf16)        # colsum partials in
            pout_d = dram.tile([2, NCORES, CLH], bf16)
            spd = dram.tile([NG, LG * PSH], bf16)    # spatial bounce

            with tc.tile_pool(name="persist", bufs=1) as persist:
                Mt = persist.tile([128, NI, PSH], bf16, name="Mt")
                U_sb = persist.tile([L, PSH], f32)
                U_pix = persist.tile([128, NJ, L], f32)
                ident = persist.tile([L, L], f32)
                make_identity(nc, ident[:])
                idnys = persist.tile([NYS, NYS], bf16)
                make_identity(nc, idnys[:])
                id128 = persist.tile([128, 128], bf16)
                make_identity(nc, id128[:])
                QRs = persist.tile([NF, PSH], bf16)
                nc.sync.dma_start(QRs[:], qrb[:, :])
                Bm = persist.tile([W, W], bf16)
                nc.sync.dma_start(Bm[:], bmat[:, :])
                By = persist.tile([H, NYS], bf16)
                nc.sync.dma_start(By[:], bmy[:, :])
                csacc = persist.tile([128, NI], f32)
                rnX = persist.tile([128, NI, L], bf16)
                rnpix = persist.tile([128, NJ], f32)

                with (
                    tc.tile_pool(name="itq", bufs=1) as itq,
                    tc.tile_pool(name="ite", bufs=3) as ite,
                    tc.tile_pool(name="spt", bufs=1) as spt,
                    tc.tile_pool(name="ittp", bufs=1, space="PSUM") as ittp,
                ):
                    # ---- prologue: U, q0, early AllGather ---------------
                    ut = itq.tile([L, PSH], f32, tag="qh")
                    nc.sync.dma_start(ut[:], unary_m[:, :])
                    nc.vector.tensor_scalar(ut[:], ut[:], 1e-5, 1.0,
                                            op0=ALU.max, op1=ALU.min)
                    nc.scalar.activation(U_sb[:], ut[:], AF.Ln)
                    e9 = ite.tile([128, NJ, L], f32, tag="e9")
                    zz = ite.tile([128, NJ], f32, tag="zz")
                    rz = ite.tile([128, NJ], f32, tag="rz")
                    tpall = ittp.tile([128, NJ * L], f32, tag="tp")
                    for jc in range(NJ):
                        tp = tpall[:, jc * L:(jc + 1) * L]
                        nc.tensor.transpose(
                            tp, U_sb[:, jc * 128:(jc + 1) * 128], ident[:])
                        nc.scalar.activation(e9[:, jc, :], tp, AF.Exp,
                                             accum_out=zz[:, jc:jc + 1])
                    nc.vector.tensor_copy(
                        U_pix[:].rearrange("a b c -> a (b c)"), tpall[:])
                    nc.vector.reciprocal(rz[:], zz[:])
                    qm0 = ite.tile([128, NJ, L], bf16, tag="qm")
                    for jc in range(NJ):
                        nc.vector.tensor_scalar_mul(
                            qm0[:, jc, :], e9[:, jc, :], rz[:, jc:jc + 1])
                    nc.gpsimd.dma_start(
                        qsh_d.rearrange("a b c -> b a c"), qm0[:])
                    nc.gpsimd.collective_compute(
                        "AllGather", mybir.AluOpType.bypass,
                        replica_groups=rg,
                        ins=[qsh_d.rearrange("a b c -> (a b c)")],
                        outs=[qfl_d.rearrange("a b -> (a b)")])

                    # ---- E = exp(T') build; halfwise colsum partial AGs -
                    cstp_cm = tc.tile_pool(name="csp", bufs=1)
                    cstp = cstp_cm.__enter__()
                    with (
                        tc.tile_pool(name="slab", bufs=2) as slabp,
                        tc.tile_pool(name="eps", bufs=2, space="PSUM") as eps,
                    ):
                        for sb in range(NSLAB):
                            sl = slabp.tile([NF, PSH], bf16, tag="sl")
                            nc.sync.dma_start(
                                sl[:], plb[:, sb * PSH:(sb + 1) * PSH])
                            for k in range(NI // NSLAB):
                                ic = sb * (NI // NSLAB) + k
                                ps = eps.tile([128, PSH], f32, tag="eps")
                                lh = sl[:, k * 128:(k + 1) * 128]
                                for (o, n) in COLS:
                                    nc.tensor.matmul(
                                        ps[:, o:o + n], lh,
                                        QRs[:, o:o + n],
                                        start=True, stop=True)
                                nc.scalar.activation(
                                    Mt[:, ic, :], ps[:], AF.Exp,
                                    accum_out=csacc[:, ic:ic + 1])
                                if ic == NIH - 1 or ic == NI - 1:
                                    hf = 0 if ic < NIH else 1
                                    io = hf * NIH
                                    csb = cstp.tile([128, NIH], bf16,
                                                    tag=f"csb{hf}")
                                    nc.vector.tensor_copy(
                                        csb[:], csacc[:, io:io + NIH])
                                    nc.gpsimd.dma_start(
                                        pin_d[hf, :].rearrange(
                                            "(p i) -> p i", p=128, i=NIH),
                                        csb[:])
                                    nc.gpsimd.collective_compute(
                                        "AllGather", mybir.AluOpType.bypass,
                                        replica_groups=rg,
                                        ins=[pin_d[hf, :]],
                                        outs=[pout_d[hf].rearrange(
                                            "a b -> (a b)")])

                        # iter-1 gather DMAs first: only depend on the
                        # q0 AllGather, must not queue behind rn staging
                        selt = cstp.tile([NI, NJ], bf16)
                        nc.sync.dma_start(selt[:], selb[:, :])
                        Qi1 = itq.tile([H, XL], bf16, tag="Qi")
                        nc.sync.dma_start(
                            Qi1[:], qfl_d.rearrange(
                                "a b -> (a b)").rearrange(
                                "(y xl) -> y xl", y=H, xl=XL))
                        qf1 = itq.tile([128, NI, L], bf16, tag="qf")
                        for c in range(NCORES):
                            nc.sync.dma_start(
                                qf1[:, c * NJ:(c + 1) * NJ, :],
                                qfl_d[c:c + 1, :].rearrange(
                                    "one (a b c2) -> (one b) a c2",
                                    a=NJ, b=128, c2=L))

                        def rn_half(hf, eng, fill_rnx=True):
                            io = hf * NIH
                            st = cstp.tile([128, NCORES, NIH], bf16,
                                           tag="st")
                            eng.dma_start(
                                st[:],
                                pout_d[hf].rearrange(
                                    "c (p i) -> p c i", p=128, i=NIH))
                            cs = cstp.tile([128, NIH], f32, tag=f"cs{hf}")
                            nc.vector.tensor_add(cs[:], st[:, 0, :],
                                                 st[:, 1, :])
                            for c in range(2, NCORES):
                                nc.vector.tensor_add(cs[:], cs[:],
                                                     st[:, c, :])
                            nc.vector.reciprocal(cs[:], cs[:])
                            nc.scalar.activation(cs[:], cs[:], AF.Sqrt)
                            if fill_rnx:
                                for l in range(L):
                                    nc.vector.tensor_copy(
                                        rnX[:, io:io + NIH, l], cs[:])
                            return cs

                        def scale_half(qf, hf):
                            for c in range(hf * 4, hf * 4 + 4):
                                cs_ = slice(c * NJ, (c + 1) * NJ)
                                nc.vector.tensor_mul(
                                    qf[:, cs_, :].rearrange(
                                        "a b c -> a (b c)"),
                                    qf[:, cs_, :].rearrange(
                                        "a b c -> a (b c)"),
                                    rnX[:, cs_, :].rearrange(
                                        "a b c -> a (b c)"))

                        csA = rn_half(0, nc.sync)
                        scale_half(qf1, 0)
                        csB = rn_half(1, nc.sync)
                        scale_half(qf1, 1)

                        # J-side rn via on-chip PE transpose of the rn
                        # vector (the transposed-layout DRAM gather costs
                        # 16us of 2-byte-run DMA scatter; this is ~6 ops).
                        # The 4x factor rides in selb (host fills 4.0).
                        rnSb = cstp.tile([128, NI], bf16)
                        nc.vector.tensor_copy(rnSb[:, 0:NIH], csA[:])
                        nc.vector.tensor_copy(rnSb[:, NIH:NI], csB[:])
                        rnTp = ittp.tile([NI, 128], bf16, tag="tp")
                        nc.tensor.transpose(rnTp[:], rnSb[:], id128[:])
                        rnT = cstp.tile([NI, 128], bf16)
                        nc.vector.tensor_copy(rnT[:], rnTp[:])

                        def rn_jside():
                            # one-hot extraction; inside iter-1 after its
                            # matvec so it does not block the PE queue
                            rnpp = ittp.tile([128, NJ], f32, tag="tp")
                            nc.tensor.matmul(rnpp[:], rnT[:], selt[:],
                                             start=True, stop=True)
                            nc.vector.tensor_copy(rnpix[:], rnpp[:])

                    # ---- iterations 1..niters --------------------------
                    with (
                        tc.tile_pool(name="itps", bufs=1,
                                     space="PSUM") as itps,
                        tc.tile_pool(name="spps", bufs=1,
                                     space="PSUM") as spps,
                    ):
                        for it in range(1, niters + 1):
                            if it == 1:
                                Qi = Qi1
                                qf = qf1
                            else:
                                Qi = itq.tile([H, XL], bf16, tag="Qi")
                                nc.sync.dma_start(
                                    Qi[:], qfl_d.rearrange(
                                        "a b -> (a b)").rearrange(
                                        "(y xl) -> y xl", y=H, xl=XL))
                                qf = itq.tile([128, NI, L], bf16, tag="qf")
                                for c in range(NCORES):
                                    cs_ = slice(c * NJ, (c + 1) * NJ)
                                    nc.sync.dma_start(
                                        qf[:, cs_, :],
                                        qfl_d[c:c + 1, :].rearrange(
                                            "one (a b c2) -> (one b) a c2",
                                            a=NJ, b=128, c2=L))
                                    nc.vector.tensor_mul(
                                        qf[:, cs_, :].rearrange(
                                            "a b c -> a (b c)"),
                                        qf[:, cs_, :].rearrange(
                                            "a b c -> a (b c)"),
                                        rnX[:, cs_, :].rearrange(
                                            "a b c -> a (b c)"))

                            # matvec with spatial ops interleaved in the
                            # PE stream; for it>=2 run bank-major so each
                            # PSUM bank's columns finalize early and the
                            # softmax overlaps the remaining banks
                            ps = itps.tile([L, PSH], f32, tag="qbps")
                            zs = spt.tile([NYS, W, L], bf16, tag="zs")
                            spx = spt.tile([128, NJ, L], bf16, tag="spx")
                            usp = spt.tile([128, NJ, L], f32, tag="usp")
                            qs = itq.tile([L, PSH], f32, tag="qh")
                            zt = spt.tile([W, NYS * L], bf16, tag="zt2")
                            sp3 = spt.tile([NYS * LG, NG, W], bf16,
                                           tag="sp3b")
                            e9 = ite.tile([128, NJ, L], f32, tag="e9")
                            zz = ite.tile([128, NJ], f32, tag="zz")
                            rz = ite.tile([128, NJ], f32, tag="rz")
                            m1 = ite.tile([128, NJ, L], f32, tag="m1")
                            tpall = ittp.tile([128, NJ * L], f32, tag="tp")

                            SL1, SL2, SL3 = (12, 38, 60)

                            def spatial_slot(ic):
                                if ic == 0:
                                    for h in range(2):
                                        zmp = spps.tile([NYS, HXL], f32,
                                                        tag="zm")
                                        b0 = h * HXL
                                        for (o, n) in ((0, 512),
                                                       (512, HXL - 512)):
                                            nc.tensor.matmul(
                                                zmp[:, o:o + n], By[:],
                                                Qi[:, b0 + o:b0 + o + n],
                                                start=True, stop=True,
                                                skip_group_check=True)
                                        nc.vector.tensor_copy(
                                            zs[:, h * (W // 2):
                                               (h + 1) * (W // 2), :]
                                            .rearrange("a b c -> a (b c)"),
                                            zmp[:])
                                if ic == SL1:
                                    ztp = spps.tile([W, NYS * L], bf16,
                                                    tag="zt")
                                    for l in range(L):
                                        nc.tensor.transpose(
                                            ztp[:, l * NYS:(l + 1) * NYS],
                                            zs[:, :, l], idnys[:])
                                    nc.vector.tensor_copy(zt[:], ztp[:])
                                if ic == SL2:
                                    sp3p = spps.tile([NYS * LG, W], f32,
                                                     tag="sp3")
                                    for g in range(NG):
                                        nc.tensor.matmul(
                                            sp3p[:],
                                            zt[:, g * NYS * LG:
                                               (g + 1) * NYS * LG],
                                            Bm[:], start=True, stop=True,
                                            skip_group_check=True)
                                        nc.vector.tensor_copy(sp3[:, g, :],
                                                              sp3p[:])
                                        nc.sync.dma_start(
                                            spd[g, :].rearrange(
                                                "(a b) -> a b",
                                                a=NYS * LG, b=W),
                                            sp3[:, g, :])
                                if ic == SL3:
                                    spdf = spd.rearrange("a b -> (a b)")
                                    for jc in range(NJ):
                                        nc.sync.dma_start(
                                            spx[:, jc, :],
                                            spdf[jc * 128:].rearrange(
                                                "(l r) -> r l",
                                                l=L, r=128)
                                            if False else
                                            spdf.rearrange(
                                                "(l p) -> l p",
                                                l=L, p=PSH)[
                                                :, jc * 128:
                                                (jc + 1) * 128].rearrange(
                                                "l r -> r l"))
                                    nc.vector.tensor_add(
                                        usp[:].rearrange("a b c -> a (b c)"),
                                        U_pix[:].rearrange(
                                            "a b c -> a (b c)"),
                                        spx[:].rearrange(
                                            "a b c -> a (b c)"))
                                return

                            def softmax_jc(jc):
                                tp = tpall[:, jc * L:(jc + 1) * L]
                                nc.tensor.transpose(
                                    tp, qs[:, jc * 128:(jc + 1) * 128],
                                    ident[:])
                                nc.vector.scalar_tensor_tensor(
                                    m1[:, jc, :], tp, rnpix[:, jc:jc + 1],
                                    usp[:, jc, :],
                                    op0=ALU.mult, op1=ALU.add)
                                nc.scalar.activation(
                                    e9[:, jc, :], m1[:, jc, :], AF.Exp,
                                    accum_out=zz[:, jc:jc + 1])

                            if it == 1:
                                # chunk-major: B-half chunks arrive late
                                for ic in range(NI):
                                    spatial_slot(ic)
                                    lhq = qf[:, ic, :]
                                    for (o, n) in COLS:
                                        nc.tensor.matmul(
                                            ps[:, o:o + n], lhq,
                                            Mt[:, ic, o:o + n],
                                            start=(ic == 0),
                                            stop=(ic == NI - 1),
                                            skip_group_check=True)
                                rn_jside()
                                nc.vector.tensor_copy(qs[:], ps[:])
                                for jc in range(NJ):
                                    softmax_jc(jc)
                            else:
                                BANKJC = ((0, 4), (4, 8), (8, 9))
                                last = (it == niters)
                                qm = ite.tile([128, NJ, L],
                                              f32 if last else bf16,
                                              tag="qm")

                                def epi_tail(bi):
                                    j0, j1 = BANKJC[bi]
                                    nc.vector.reciprocal(rz[:, j0:j1],
                                                         zz[:, j0:j1])
                                    for jc in range(j0, j1):
                                        nc.vector.tensor_scalar_mul(
                                            qm[:, jc, :], e9[:, jc, :],
                                            rz[:, jc:jc + 1])
                                    if not last:
                                        nc.sync.dma_start(
                                            qsh_d[j0:j1].rearrange(
                                                "a b c -> b a c"),
                                            qm[:, j0:j1, :])

                                def epi_bank(bi):
                                    j0, j1 = BANKJC[bi]
                                    for jc in range(j0, j1):
                                        softmax_jc(jc)
                                    epi_tail(bi)

                                for bi, (o, n) in enumerate(COLS):
                                    for ic in range(NI):
                                        if bi == 0:
                                            spatial_slot(ic)
                                        nc.tensor.matmul(
                                            ps[:, o:o + n], qf[:, ic, :],
                                            Mt[:, ic, o:o + n],
                                            start=(ic == 0),
                                            stop=(ic == NI - 1),
                                            skip_group_check=True)
                                    nc.vector.tensor_copy(
                                        qs[:, o:o + n], ps[:, o:o + n])
                                    if bi > 0:
                                        epi_bank(bi - 1)
                                epi_bank(2)
                                if not last:
                                    nc.gpsimd.collective_compute(
                                        "AllGather", mybir.AluOpType.bypass,
                                        replica_groups=rg,
                                        ins=[qsh_d.rearrange(
                                            "a b c -> (a b c)")],
                                        outs=[qfl_d.rearrange(
                                            "a b -> (a b)")])
                                else:
                                    nc.sync.dma_start(
                                        qout[:, :, :].rearrange(
                                            "a b c -> b a c"), qm[:])
                                continue
                            nc.vector.reciprocal(rz[:], zz[:])
                            qm = ite.tile([128, NJ, L],
                                          f32 if it == niters else bf16,
                                          tag="qm")
                            for jc in range(NJ):
                                nc.vector.tensor_scalar_mul(
                                    qm[:, jc, :], e9[:, jc, :],
                                    rz[:, jc:jc + 1])
                            if it < niters:
                                nc.sync.dma_start(
                                    qsh_d.rearrange("a b c -> b a c"), qm[:])
                                nc.gpsimd.collective_compute(
                                    "AllGather", mybir.AluOpType.bypass,
                                    replica_groups=rg,
                                    ins=[qsh_d.rearrange(
                                        "a b c -> (a b c)")],
                                    outs=[qfl_d.rearrange("a b -> (a b)")])
                            else:
                                nc.sync.dma_start(
                                    qout[:, :, :].rearrange("a b c -> b a c"),
                                    qm[:])
                        cstp_cm.__exit__(None, None, None)

    nc.finalize()
    _CACHE[key] = nc
    return nc


# ----------------------------------------------------------------------------
# host entry point
# ----------------------------------------------------------------------------

def _in_maps(unary, ref):
    B, _, _ = _host_constants()
    PL, QR = _feature_rows(ref)
    u2 = np.ascontiguousarray(np.asarray(unary, np.float32).reshape(L, P))
    bmat = np.ascontiguousarray(B.astype(_bf16))
    maps = []
    for c in range(NCORES):
        sl = slice(c * PSH, (c + 1) * PSH)
        sel = np.zeros((NI, NJ), _bf16)
        for jc in range(NJ):
            sel[c * NJ + jc, jc] = 4.0   # folds COMPAT_BF into rn_j
        bmyc = np.ascontiguousarray(
            (2.0 * B[:, c * NYS:(c + 1) * NYS]).astype(_bf16))
        maps.append({
            "unary_m": np.ascontiguousarray(u2[:, sl]),
            "plb": PL,
            "qrb": np.ascontiguousarray(QR[:, sl]),
            "bmat": bmat,
            "bmy": bmyc,
            "selb": sel,
        })
    return maps


def kernel(unary: np.ndarray, ref: np.ndarray) -> np.ndarray:
    from concourse import bass_utils

    nc = _build_bass()
    in_maps = _in_maps(unary, ref)

    global LAST_RESULT
    res = bass_utils.run_bass_kernel_spmd(nc, in_maps,
                                          core_ids=list(range(NCORES)),
                                          trace=TRACE)
    LAST_RESULT = res
    shards = [res.results[c]["qout"].reshape(PSH, L) for c in range(NCORES)]
    qfull = np.concatenate(shards, 0)          # [P, L]
    out = qfull.T.reshape(1, L, H, W).astype(np.float32)
    return out


if __name__ == "__main__":
    u = np.random.rand(1, L, H, W).astype(np.float32)
    r = (np.random.rand(1, 3, H, W) * 255).astype(np.float32)
    o = kernel(u, r)
    print(o.shape, o.dtype, o.sum())



# revision 12
# speedup vs baseline: 3.3942x; 3.3942x over previous
"""DenseCRF mean-field inference on 8 Trainium2 NeuronCores.

Math: per image, 5 iterations of
    q_hat = U + 4*((q/n) @ K)/n + 2*(q @ S);  q = softmax(q_hat, axis=0)
with K[i,j] = exp(-0.5*d2(i,j)) the dense 9216x9216 bilateral kernel,
n = sqrt(K @ 1), S = kron(B,B) the separable (normalized, banded) 71x71
spatial kernel.

Sharding: K's columns over 8 cores; each core holds K[:, mine] =
[9216, 1152] bf16 (~21 MB) resident in SBUF. Per iteration each core
computes its 1152 columns of the update, softmaxes them pixel-major, and
an AllGather redistributes the full q.

Pipeline structure (driven by the TimelineSim cost model, 757us -> 495us):
  * Build: E=exp(T') via bf16 feature matmuls -- exact hi/lo split of the
    color features (products of bf16 pairs are exact in f32 PSUM, so d2
    carries no cancellation error) -- with ACT accum_out producing
    colsum(K) partials for free (rowsum of my block = colsum restricted
    to my j's, by K's symmetry). ACT-exp-bound at ~105us.
  * q0 = softmax(U) AllGathers at ~6us, fully hidden inside the build;
    the colsum partials AllGather in two halves (the first fires
    mid-build) so iteration 1's first 36 chunks start right at build end.
  * No M-rescale pass: the gathered q is scaled by rn_i in place (per-core
    slices as each gather DMA lands), rn_j folds into the pixel-major
    softmax input (scalar_tensor_tensor with a per-partition rn), and the
    +2S spatial term is computed separably each iteration -- stage-1
    matmul vs Bmy = 2*B[:,mine], 21 strided PE transposes, 3 class-batched
    stage-2 matmuls vs B, all interleaved into the matvec's PE stream,
    with a contiguous DRAM bounce landing sp pixel-major.
  * Iterations >= 2 run the matvec bank-major (3 PSUM column banks x 72
    chunks) so each bank's columns finalize early and the softmax of bank
    b-1 overlaps bank b's matmuls; per-bank reciprocal/scale/store lets
    the AllGather trigger right after the last bank's short chain.
  * rn extraction for the SPMD-uniform program: a one-hot selector matmul
    (per-core input) pulls the core's own 4*rn_j out of the transposed
    colsum pipeline (a program cannot slice by core id at compile time).
"""

import numpy as np
import ml_dtypes

H = 96
W = 96
P = H * W            # 9216 pixels
L = 21               # classes
NCORES = 8
PSH = P // NCORES    # 1152 pixels per core
NI = P // 128        # 72 contraction chunks
NJ = PSH // 128      # 9 output-pixel chunks per core
NYS = PSH // W       # 12 image rows per core
NSLAB = 8            # lhsT feature slabs of 1152 columns
NF = 15              # extended feature rows
NG = 3               # stage-2 class groups (7 classes each)
LPAD = 32            # DoubleRow stationary free dim (>=32), classes padded
LG = L // NG
SXY_BF = 70.0
SC_BF = 12.0
LN4 = float(np.log(4.0))
QLEN = NJ * 128 * L      # 24192 bf16 per q shard
CLEN = 128 * NI          # 9216 colsum partials
XL = W * L               # 2016

_bf16 = ml_dtypes.bfloat16

_CACHE = {}
TRACE = False      # set by test harness for profiling runs
LAST_RESULT = None


# ----------------------------------------------------------------------------
# host-side input prep
# ----------------------------------------------------------------------------

def _host_constants():
    if "consts" in _CACHE:
        return _CACHE["consts"]
    # 1D spatial gaussian band matrix B[a,b] = exp(-(a-b)^2/72)/z, |a-b|<=35
    sig_sq = 36.0
    rr = 35
    g1 = np.exp(-((np.arange(2 * rr + 1, dtype=np.float64) - rr) ** 2)
                / (2 * sig_sq))
    z = g1.sum()
    idx = np.arange(H)
    d = idx[:, None] - idx[None, :]
    B = np.where(np.abs(d) <= rr,
                 np.exp(-(d.astype(np.float64) ** 2) / (2 * sig_sq)) / z, 0.0)
    ys = np.arange(P) // W
    xs = np.arange(P) % W
    _CACHE["consts"] = (B, ys, xs)
    return _CACHE["consts"]


def _feature_rows(ref):
    """15 extended-feature rows, PL [15,P] (lhsT side) / QR [15,P] (rhs)."""
    _, ys, xs = _host_constants()
    r2 = np.asarray(ref, np.float64).reshape(3, P)
    yf = (ys / SXY_BF).astype(_bf16)
    xf = (xs / SXY_BF).astype(_bf16)
    ch = (r2 / SC_BF).astype(_bf16)                       # [3, P]
    cl = (r2 / SC_BF - ch.astype(np.float64)).astype(_bf16)
    fy = yf.astype(np.float64)
    fx = xf.astype(np.float64)
    fc = ch.astype(np.float64) + cl.astype(np.float64)
    dvec = -0.5 * (fy ** 2 + fx ** 2 + (fc ** 2).sum(0))  # [P]
    dh = dvec.astype(_bf16)
    dl = (dvec - dh.astype(np.float64)).astype(_bf16)
    ones = np.ones(P, _bf16)
    PL = np.stack([yf, xf, ch[0], ch[1], ch[2], ch[0], ch[1], ch[2],
                   cl[0], cl[1], cl[2], dh, dl, ones, ones], 0)
    QR = np.stack([yf, xf, ch[0], ch[1], ch[2], cl[0], cl[1], cl[2],
                   ch[0], ch[1], ch[2], ones, ones, dh, dl], 0)
    return np.ascontiguousarray(PL), np.ascontiguousarray(QR)


# ----------------------------------------------------------------------------
# device program
# ----------------------------------------------------------------------------

def _build_bass(niters=5):
    key = ("nc", niters)
    if key in _CACHE:
        return _CACHE[key]

    import concourse.bass as bass
    import concourse.bacc as bacc
    import concourse.tile as tile
    import concourse.mybir as mybir
    from concourse.masks import make_identity

    f32 = mybir.dt.float32
    bf16 = mybir.dt.bfloat16
    fp8 = mybir.dt.float8e4
    DR = mybir.MatmulPerfMode.DoubleRow
    AF = mybir.ActivationFunctionType
    ALU = mybir.AluOpType

    nc = bacc.Bacc("TRN2", num_devices=NCORES)

    unary_m = nc.dram_tensor("unary_m", [L, PSH], f32, kind="ExternalInput")
    plb = nc.dram_tensor("plb", [NF, P], bf16, kind="ExternalInput")
    qrb = nc.dram_tensor("qrb", [NF, PSH], bf16, kind="ExternalInput")
    bmat = nc.dram_tensor("bmat", [W, W], bf16, kind="ExternalInput")
    bmy = nc.dram_tensor("bmy", [H, NYS], bf16, kind="ExternalInput")
    selb = nc.dram_tensor("selb", [NI, NJ], bf16, kind="ExternalInput")
    qout = nc.dram_tensor("qout", [NJ, 128, L], f32, kind="ExternalOutput")

    rg = [list(range(NCORES))]
    COLS = ((0, 512), (512, 512), (1024, 128))
    # fp8 DoubleRow matvec: 2 contraction k-tiles per instruction, moving
    # free dim 2n <= 512 so column groups of <= 256; grouped to the same
    # 3 PSUM bank ranges as COLS for the bank-major epilogue pipeline.
    BANKCOLS = (((0, 256), (256, 256)), ((512, 256), (768, 256)),
                ((1024, 128),))
    HXL = XL // 2
    NIH = NI // 2          # 36: partials-AG half
    CLH = CLEN // 2

    with tile.TileContext(nc) as tc:
        with tc.tile_pool(name="dram", bufs=1, space="DRAM") as dram:
            qsh_d = dram.tile([NJ, 128, L], bf16)    # my q shard, AG input
            qfl_d = dram.tile([NCORES, QLEN], bf16)  # AG output
            pin_d = dram.tile([2, CLH], bf16)        # colsum partials in
            pout_d = dram.tile([2, NCORES, CLH], bf16)
            spd = dram.tile([NG, LG * PSH], bf16)    # spatial bounce

            with tc.tile_pool(name="persist", bufs=1) as persist:
                Mt = persist.tile([128, NI, PSH], fp8, name="Mt")
                U_sb = persist.tile([L, PSH], f32)
                U_pix = persist.tile([128, NJ, L], f32)
                ident = persist.tile([L, L], f32)
                make_identity(nc, ident[:])
                idnys = persist.tile([NYS, NYS], bf16)
                make_identity(nc, idnys[:])
                id128 = persist.tile([128, 128], bf16)
                make_identity(nc, id128[:])
                QRs = persist.tile([NF, PSH], bf16)
                nc.sync.dma_start(QRs[:], qrb[:, :])
                Bm = persist.tile([W, W], bf16)
                nc.sync.dma_start(Bm[:], bmat[:, :])
                By = persist.tile([H, NYS], bf16)
                nc.sync.dma_start(By[:], bmy[:, :])
                csacc = persist.tile([128, NI], f32)
                rnX = persist.tile([128, NI, L], bf16)
                rnpix = persist.tile([128, NJ], f32)

                with (
                    tc.tile_pool(name="itq", bufs=1) as itq,
                    tc.tile_pool(name="ite", bufs=3) as ite,
                    tc.tile_pool(name="spt", bufs=1) as spt,
                    tc.tile_pool(name="ittp", bufs=1, space="PSUM") as ittp,
                ):
                    # ---- prologue: U, q0, early AllGather ---------------
                    ut = itq.tile([L, PSH], f32, tag="qh")
                    nc.sync.dma_start(ut[:], unary_m[:, :])
                    nc.vector.tensor_scalar(ut[:], ut[:], 1e-5, 1.0,
                                            op0=ALU.max, op1=ALU.min)
                    nc.scalar.activation(U_sb[:], ut[:], AF.Ln)
                    e9 = ite.tile([128, NJ, L], f32, tag="e9")
                    zz = ite.tile([128, NJ], f32, tag="zz")
                    rz = ite.tile([128, NJ], f32, tag="rz")
                    tpall = ittp.tile([128, NJ * L], f32, tag="tp")
                    for jc in range(NJ):
                        tp = tpall[:, jc * L:(jc + 1) * L]
                        nc.tensor.transpose(
                            tp, U_sb[:, jc * 128:(jc + 1) * 128], ident[:])
                        nc.scalar.activation(e9[:, jc, :], tp, AF.Exp,
                                             accum_out=zz[:, jc:jc + 1])
                    nc.vector.tensor_copy(
                        U_pix[:].rearrange("a b c -> a (b c)"), tpall[:])
                    nc.vector.reciprocal(rz[:], zz[:])
                    qm0 = ite.tile([128, NJ, L], bf16, tag="qm")
                    for jc in range(NJ):
                        nc.vector.tensor_scalar_mul(
                            qm0[:, jc, :], e9[:, jc, :], rz[:, jc:jc + 1])
                    nc.gpsimd.dma_start(
                        qsh_d.rearrange("a b c -> b a c"), qm0[:])
                    nc.gpsimd.collective_compute(
                        "AllGather", mybir.AluOpType.bypass,
                        replica_groups=rg,
                        ins=[qsh_d.rearrange("a b c -> (a b c)")],
                        outs=[qfl_d.rearrange("a b -> (a b)")])

                    # ---- E = exp(T') build; halfwise colsum partial AGs -
                    cstp_cm = tc.tile_pool(name="csp", bufs=1)
                    cstp = cstp_cm.__enter__()
                    with (
                        tc.tile_pool(name="slab", bufs=2) as slabp,
                        tc.tile_pool(name="eps", bufs=2, space="PSUM") as eps,
                    ):
                        for sb in range(NSLAB):
                            sl = slabp.tile([NF, PSH], bf16, tag="sl")
                            nc.sync.dma_start(
                                sl[:], plb[:, sb * PSH:(sb + 1) * PSH])
                            for k in range(NI // NSLAB):
                                ic = sb * (NI // NSLAB) + k
                                ps = eps.tile([128, PSH], f32, tag="eps")
                                lh = sl[:, k * 128:(k + 1) * 128]
                                for (o, n) in COLS:
                                    nc.tensor.matmul(
                                        ps[:, o:o + n], lh,
                                        QRs[:, o:o + n],
                                        start=True, stop=True)
                                nc.scalar.activation(
                                    Mt[:, ic, :], ps[:], AF.Exp,
                                    accum_out=csacc[:, ic:ic + 1])
                                if ic == NIH - 1 or ic == NI - 1:
                                    hf = 0 if ic < NIH else 1
                                    io = hf * NIH
                                    csb = cstp.tile([128, NIH], bf16,
                                                    tag=f"csb{hf}")
                                    nc.vector.tensor_copy(
                                        csb[:], csacc[:, io:io + NIH])
                                    nc.gpsimd.dma_start(
                                        pin_d[hf, :].rearrange(
                                            "(p i) -> p i", p=128, i=NIH),
                                        csb[:])
                                    nc.gpsimd.collective_compute(
                                        "AllGather", mybir.AluOpType.bypass,
                                        replica_groups=rg,
                                        ins=[pin_d[hf, :]],
                                        outs=[pout_d[hf].rearrange(
                                            "a b -> (a b)")])

                        # iter-1 gather DMAs first: only depend on the
                        # q0 AllGather, must not queue behind rn staging
                        selt = cstp.tile([NI, NJ], bf16)
                        nc.sync.dma_start(selt[:], selb[:, :])
                        Qi1 = itq.tile([H, XL], bf16, tag="Qi")
                        nc.sync.dma_start(
                            Qi1[:], qfl_d.rearrange(
                                "a b -> (a b)").rearrange(
                                "(y xl) -> y xl", y=H, xl=XL))
                        qf1 = itq.tile([128, NI, L], bf16, tag="qf")
                        qf18 = itq.tile([128, NI, LPAD], fp8, tag="qf8")
                        nc.vector.memset(qf18[:, :, L:LPAD], 0.0)
                        for c in range(NCORES):
                            nc.sync.dma_start(
                                qf1[:, c * NJ:(c + 1) * NJ, :],
                                qfl_d[c:c + 1, :].rearrange(
                                    "one (a b c2) -> (one b) a c2",
                                    a=NJ, b=128, c2=L))

                        def rn_half(hf, eng, fill_rnx=True):
                            io = hf * NIH
                            st = cstp.tile([128, NCORES, NIH], bf16,
                                           tag="st")
                            eng.dma_start(
                                st[:],
                                pout_d[hf].rearrange(
                                    "c (p i) -> p c i", p=128, i=NIH))
                            cs = cstp.tile([128, NIH], f32, tag=f"cs{hf}")
                            nc.vector.tensor_add(cs[:], st[:, 0, :],
                                                 st[:, 1, :])
                            for c in range(2, NCORES):
                                nc.vector.tensor_add(cs[:], cs[:],
                                                     st[:, c, :])
                            nc.vector.reciprocal(cs[:], cs[:])
                            nc.scalar.activation(cs[:], cs[:], AF.Sqrt)
                            if fill_rnx:
                                for l in range(L):
                                    nc.vector.tensor_copy(
                                        rnX[:, io:io + NIH, l], cs[:])
                            return cs

                        def scale_half(qf, hf):
                            for c in range(hf * 4, hf * 4 + 4):
                                cs_ = slice(c * NJ, (c + 1) * NJ)
                                nc.vector.tensor_mul(
                                    qf18[:, cs_, 0:L],
                                    qf[:, cs_, :],
                                    rnX[:, cs_, :])

                        csA = rn_half(0, nc.sync)
                        scale_half(qf1, 0)
                        csB = rn_half(1, nc.sync)
                        scale_half(qf1, 1)

                        # J-side rn via on-chip PE transpose of the rn
                        # vector (the transposed-layout DRAM gather costs
                        # 16us of 2-byte-run DMA scatter; this is ~6 ops).
                        # The 4x factor rides in selb (host fills 4.0).
                        rnSb = cstp.tile([128, NI], bf16)
                        nc.vector.tensor_copy(rnSb[:, 0:NIH], csA[:])
                        nc.vector.tensor_copy(rnSb[:, NIH:NI], csB[:])
                        rnTp = ittp.tile([NI, 128], bf16, tag="tp")
                        nc.tensor.transpose(rnTp[:], rnSb[:], id128[:])
                        rnT = cstp.tile([NI, 128], bf16)
                        nc.vector.tensor_copy(rnT[:], rnTp[:])

                        def rn_jside():
                            # one-hot extraction; inside iter-1 after its
                            # matvec so it does not block the PE queue
                            rnpp = ittp.tile([128, NJ], f32, tag="tp")
                            nc.tensor.matmul(rnpp[:], rnT[:], selt[:],
                                             start=True, stop=True)
                            nc.vector.tensor_copy(rnpix[:], rnpp[:])

                    # ---- iterations 1..niters --------------------------
                    with (
                        tc.tile_pool(name="itps", bufs=1,
                                     space="PSUM") as itps,
                        tc.tile_pool(name="spps", bufs=1,
                                     space="PSUM") as spps,
                    ):
                        for it in range(1, niters + 1):
                            if it == 1:
                                Qi = Qi1
                                qf8 = qf18
                            else:
                                Qi = itq.tile([H, XL], bf16, tag="Qi")
                                nc.sync.dma_start(
                                    Qi[:], qfl_d.rearrange(
                                        "a b -> (a b)").rearrange(
                                        "(y xl) -> y xl", y=H, xl=XL))
                                qf = itq.tile([128, NI, L], bf16, tag="qf")
                                qf8 = itq.tile([128, NI, LPAD], fp8,
                                                tag="qf8")
                                for c in range(NCORES):
                                    cs_ = slice(c * NJ, (c + 1) * NJ)
                                    nc.sync.dma_start(
                                        qf[:, cs_, :],
                                        qfl_d[c:c + 1, :].rearrange(
                                            "one (a b c2) -> (one b) a c2",
                                            a=NJ, b=128, c2=L))
                                    nc.vector.tensor_mul(
                                        qf8[:, cs_, 0:L],
                                        qf[:, cs_, :],
                                        rnX[:, cs_, :])

                            # matvec with spatial ops interleaved in the
                            # PE stream; for it>=2 run bank-major so each
                            # PSUM bank's columns finalize early and the
                            # softmax overlaps the remaining banks
                            ps = itps.tile([LPAD, PSH], f32, tag="qbps")
                            zs = spt.tile([NYS, W, L], bf16, tag="zs")
                            spx = spt.tile([128, NJ, L], bf16, tag="spx")
                            usp = spt.tile([128, NJ, L], f32, tag="usp")
                            qs = itq.tile([L, PSH], f32, tag="qh")
                            zt = spt.tile([W, NYS * L], bf16, tag="zt2")
                            sp3 = spt.tile([NYS * LG, NG, W], bf16,
                                           tag="sp3b")
                            e9 = ite.tile([128, NJ, L], f32, tag="e9")
                            zz = ite.tile([128, NJ], f32, tag="zz")
                            rz = ite.tile([128, NJ], f32, tag="rz")
                            m1 = ite.tile([128, NJ, L], f32, tag="m1")
                            tpall = ittp.tile([128, NJ * L], f32, tag="tp")

                            SL1, SL2, SL3 = (12, 38, 60)

                            def spatial_slot(ic):
                                if ic == 0:
                                    for h in range(2):
                                        zmp = spps.tile([NYS, HXL], f32,
                                                        tag="zm")
                                        b0 = h * HXL
                                        for (o, n) in ((0, 512),
                                                       (512, HXL - 512)):
                                            nc.tensor.matmul(
                                                zmp[:, o:o + n], By[:],
                                                Qi[:, b0 + o:b0 + o + n],
                                                start=True, stop=True,
                                                skip_group_check=True)
                                        nc.vector.tensor_copy(
                                            zs[:, h * (W // 2):
                                               (h + 1) * (W // 2), :]
                                            .rearrange("a b c -> a (b c)"),
                                            zmp[:])
                                if ic == SL1:
                                    ztp = spps.tile([W, NYS * L], bf16,
                                                    tag="zt")
                                    for l in range(L):
                                        nc.tensor.transpose(
                                            ztp[:, l * NYS:(l + 1) * NYS],
                                            zs[:, :, l], idnys[:])
                                    nc.vector.tensor_copy(zt[:], ztp[:])
                                if ic == SL2:
                                    sp3p = spps.tile([NYS * LG, W], f32,
                                                     tag="sp3")
                                    for g in range(NG):
                                        nc.tensor.matmul(
                                            sp3p[:],
                                            zt[:, g * NYS * LG:
                                               (g + 1) * NYS * LG],
                                            Bm[:], start=True, stop=True,
                                            skip_group_check=True)
                                        nc.vector.tensor_copy(sp3[:, g, :],
                                                              sp3p[:])
                                        nc.sync.dma_start(
                                            spd[g, :].rearrange(
                                                "(a b) -> a b",
                                                a=NYS * LG, b=W),
                                            sp3[:, g, :])
                                if ic == SL3:
                                    spdf = spd.rearrange("a b -> (a b)")
                                    for jc in range(NJ):
                                        nc.sync.dma_start(
                                            spx[:, jc, :],
                                            spdf[jc * 128:].rearrange(
                                                "(l r) -> r l",
                                                l=L, r=128)
                                            if False else
                                            spdf.rearrange(
                                                "(l p) -> l p",
                                                l=L, p=PSH)[
                                                :, jc * 128:
                                                (jc + 1) * 128].rearrange(
                                                "l r -> r l"))
                                    nc.vector.tensor_add(
                                        usp[:].rearrange("a b c -> a (b c)"),
                                        U_pix[:].rearrange(
                                            "a b c -> a (b c)"),
                                        spx[:].rearrange(
                                            "a b c -> a (b c)"))
                                return

                            def softmax_jc(jc):
                                tp = tpall[:, jc * L:(jc + 1) * L]
                                nc.tensor.transpose(
                                    tp, qs[:, jc * 128:(jc + 1) * 128],
                                    ident[:])
                                nc.vector.scalar_tensor_tensor(
                                    m1[:, jc, :], tp, rnpix[:, jc:jc + 1],
                                    usp[:, jc, :],
                                    op0=ALU.mult, op1=ALU.add)
                                nc.scalar.activation(
                                    e9[:, jc, :], m1[:, jc, :], AF.Exp,
                                    accum_out=zz[:, jc:jc + 1])

                            if it == 1:
                                # chunk-major: B-half chunks arrive late
                                for ic2 in range(0, NI, 2):
                                    spatial_slot(ic2)
                                    spatial_slot(ic2 + 1)
                                    for bcols in BANKCOLS:
                                        for (o, n) in bcols:
                                            nc.tensor.matmul(
                                                ps[:, o:o + n],
                                                qf8[:, ic2:ic2 + 2, :],
                                                Mt[:, ic2:ic2 + 2, o:o + n],
                                                start=(ic2 == 0),
                                                stop=(ic2 == NI - 2),
                                                perf_mode=DR,
                                                skip_group_check=True)
                                rn_jside()
                                nc.vector.tensor_copy(qs[:], ps[0:L, :])
                                for jc in range(NJ):
                                    softmax_jc(jc)
                            else:
                                BANKJC = ((0, 4), (4, 8), (8, 9))
                                last = (it == niters)
                                qm = ite.tile([128, NJ, L],
                                              f32 if last else bf16,
                                              tag="qm")

                                def epi_tail(bi):
                                    j0, j1 = BANKJC[bi]
                                    nc.vector.reciprocal(rz[:, j0:j1],
                                                         zz[:, j0:j1])
                                    for jc in range(j0, j1):
                                        nc.vector.tensor_scalar_mul(
                                            qm[:, jc, :], e9[:, jc, :],
                                            rz[:, jc:jc + 1])
                                    if not last:
                                        nc.sync.dma_start(
                                            qsh_d[j0:j1].rearrange(
                                                "a b c -> b a c"),
                                            qm[:, j0:j1, :])

                                def epi_bank(bi):
                                    j0, j1 = BANKJC[bi]
                                    for jc in range(j0, j1):
                                        softmax_jc(jc)
                                    epi_tail(bi)

                                for bi, (o, n) in enumerate(COLS):
                                    for ic2 in range(0, NI, 2):
                                        if bi == 0:
                                            spatial_slot(ic2)
                                            spatial_slot(ic2 + 1)
                                        for (o2, n2) in BANKCOLS[bi]:
                                            nc.tensor.matmul(
                                                ps[:, o2:o2 + n2],
                                                qf8[:, ic2:ic2 + 2, :],
                                                Mt[:, ic2:ic2 + 2,
                                                   o2:o2 + n2],
                                                start=(ic2 == 0),
                                                stop=(ic2 == NI - 2),
                                                perf_mode=DR,
                                                skip_group_check=True)
                                    nc.vector.tensor_copy(
                                        qs[:, o:o + n], ps[0:L, o:o + n])
                                    if bi > 0:
                                        epi_bank(bi - 1)
                                epi_bank(2)
                                if not last:
                                    nc.gpsimd.collective_compute(
                                        "AllGather", mybir.AluOpType.bypass,
                                        replica_groups=rg,
                                        ins=[qsh_d.rearrange(
                                            "a b c -> (a b c)")],
                                        outs=[qfl_d.rearrange(
                                            "a b -> (a b)")])
                                else:
                                    nc.sync.dma_start(
                                        qout[:, :, :].rearrange(
                                            "a b c -> b a c"), qm[:])
                                continue
                            nc.vector.reciprocal(rz[:], zz[:])
                            qm = ite.tile([128, NJ, L],
                                          f32 if it == niters else bf16,
                                          tag="qm")
                            for jc in range(NJ):
                                nc.vector.tensor_scalar_mul(
                                    qm[:, jc, :], e9[:, jc, :],
                                    rz[:, jc:jc + 1])
                            if it < niters:
                                nc.sync.dma_start(
                                    qsh_d.rearrange("a b c -> b a c"), qm[:])
                                nc.gpsimd.collective_compute(
                                    "AllGather", mybir.AluOpType.bypass,
                                    replica_groups=rg,
                                    ins=[qsh_d.rearrange(
                                        "a b c -> (a b c)")],
                                    outs=[qfl_d.rearrange("a b -> (a b)")])
                            else:
                                nc.sync.dma_start(
                                    qout[:, :, :].rearrange("a b c -> b a c"),
                                    qm[:])
                        cstp_cm.__exit__(None, None, None)

    nc.finalize()
    _CACHE[key] = nc
    return nc


# ----------------------------------------------------------------------------
# host entry point
# ----------------------------------------------------------------------------

def _in_maps(unary, ref):
    B, _, _ = _host_constants()
    PL, QR = _feature_rows(ref)
    u2 = np.ascontiguousarray(np.asarray(unary, np.float32).reshape(L, P))
    bmat = np.ascontiguousarray(B.astype(_bf16))
    maps = []
    for c in range(NCORES):
        sl = slice(c * PSH, (c + 1) * PSH)
        sel = np.zeros((NI, NJ), _bf16)
        for jc in range(NJ):
            sel[c * NJ + jc, jc] = 4.0   # folds COMPAT_BF into rn_j
        bmyc = np.ascontiguousarray(
            (2.0 * B[:, c * NYS:(c + 1) * NYS]).astype(_bf16))
        maps.append({
            "unary_m": np.ascontiguousarray(u2[:, sl]),
            "plb": PL,
            "qrb": np.ascontiguousarray(QR[:, sl]),
            "bmat": bmat,
            "bmy": bmyc,
            "selb": sel,
        })
    return maps


def kernel(unary: np.ndarray, ref: np.ndarray) -> np.ndarray:
    from concourse import bass_utils

    nc = _build_bass()
    in_maps = _in_maps(unary, ref)

    global LAST_RESULT
    res = bass_utils.run_bass_kernel_spmd(nc, in_maps,
                                          core_ids=list(range(NCORES)),
                                          trace=TRACE)
    LAST_RESULT = res
    shards = [res.results[c]["qout"].reshape(PSH, L) for c in range(NCORES)]
    qfull = np.concatenate(shards, 0)          # [P, L]
    out = qfull.T.reshape(1, L, H, W).astype(np.float32)
    return out


if __name__ == "__main__":
    u = np.random.rand(1, L, H, W).astype(np.float32)
    r = (np.random.rand(1, 3, H, W) * 255).astype(np.float32)
    o = kernel(u, r)
    print(o.shape, o.dtype, o.sum())



# revision 16
# speedup vs baseline: 3.8698x; 1.1401x over previous
"""DenseCRF mean-field inference on 8 Trainium2 NeuronCores.

Math: per image, 5 iterations of
    q_hat = U + 4*((q/n) @ K)/n + 2*(q @ S);  q = softmax(q_hat, axis=0)
with K[i,j] = exp(-0.5*d2(i,j)) the dense 9216x9216 bilateral kernel,
n = sqrt(K @ 1), S = kron(B,B) the separable (normalized, banded) 71x71
spatial kernel.

Sharding: K's columns over 8 cores; each core holds K[:, mine] =
[9216, 1152] bf16 (~21 MB) resident in SBUF. Per iteration each core
computes its 1152 columns of the update, softmaxes them pixel-major, and
an AllGather redistributes the full q.

Pipeline structure (driven by the TimelineSim cost model, 757us -> 495us):
  * Build: E=exp(T') via bf16 feature matmuls -- exact hi/lo split of the
    color features (products of bf16 pairs are exact in f32 PSUM, so d2
    carries no cancellation error) -- with ACT accum_out producing
    colsum(K) partials for free (rowsum of my block = colsum restricted
    to my j's, by K's symmetry). ACT-exp-bound at ~105us.
  * q0 = softmax(U) AllGathers at ~6us, fully hidden inside the build;
    the colsum partials AllGather in two halves (the first fires
    mid-build) so iteration 1's first 36 chunks start right at build end.
  * No M-rescale pass: the gathered q is scaled by rn_i in place (per-core
    slices as each gather DMA lands), rn_j folds into the pixel-major
    softmax input (scalar_tensor_tensor with a per-partition rn), and the
    +2S spatial term is computed separably each iteration -- stage-1
    matmul vs Bmy = 2*B[:,mine], 21 strided PE transposes, 3 class-batched
    stage-2 matmuls vs B, all interleaved into the matvec's PE stream,
    with a contiguous DRAM bounce landing sp pixel-major.
  * Iterations >= 2 run the matvec bank-major (3 PSUM column banks x 72
    chunks) so each bank's columns finalize early and the softmax of bank
    b-1 overlaps bank b's matmuls; per-bank reciprocal/scale/store lets
    the AllGather trigger right after the last bank's short chain.
  * rn extraction for the SPMD-uniform program: a one-hot selector matmul
    (per-core input) pulls the core's own 4*rn_j out of the transposed
    colsum pipeline (a program cannot slice by core id at compile time).
"""

import numpy as np
import ml_dtypes

H = 96
W = 96
P = H * W            # 9216 pixels
L = 21               # classes
NCORES = 8
PSH = P // NCORES    # 1152 pixels per core
NI = P // 128        # 72 contraction chunks
NJ = PSH // 128      # 9 output-pixel chunks per core
NYS = PSH // W       # 12 image rows per core
NSLAB = 8            # lhsT feature slabs of 1152 columns
NF = 15              # extended feature rows
NG = 3               # stage-2 class groups (7 classes each)
LPAD = 32            # DoubleRow stationary free dim (>=32), classes padded
LG = L // NG
SXY_BF = 70.0
SC_BF = 12.0
LN4 = float(np.log(4.0))
QLEN = NJ * 128 * L      # 24192 bf16 per q shard
CLEN = 128 * NI          # 9216 colsum partials
XL = W * L               # 2016

_bf16 = ml_dtypes.bfloat16

_CACHE = {}
TRACE = False      # set by test harness for profiling runs
LAST_RESULT = None


# ----------------------------------------------------------------------------
# host-side input prep
# ----------------------------------------------------------------------------

def _host_constants():
    if "consts" in _CACHE:
        return _CACHE["consts"]
    # 1D spatial gaussian band matrix B[a,b] = exp(-(a-b)^2/72)/z, |a-b|<=35
    sig_sq = 36.0
    rr = 35
    g1 = np.exp(-((np.arange(2 * rr + 1, dtype=np.float64) - rr) ** 2)
                / (2 * sig_sq))
    z = g1.sum()
    idx = np.arange(H)
    d = idx[:, None] - idx[None, :]
    B = np.where(np.abs(d) <= rr,
                 np.exp(-(d.astype(np.float64) ** 2) / (2 * sig_sq)) / z, 0.0)
    ys = np.arange(P) // W
    xs = np.arange(P) % W
    _CACHE["consts"] = (B, ys, xs)
    return _CACHE["consts"]


def _feature_rows(ref):
    """15 extended-feature rows, PL [15,P] (lhsT side) / QR [15,P] (rhs)."""
    _, ys, xs = _host_constants()
    r2 = np.asarray(ref, np.float64).reshape(3, P)
    yf = (ys / SXY_BF).astype(_bf16)
    xf = (xs / SXY_BF).astype(_bf16)
    ch = (r2 / SC_BF).astype(_bf16)                       # [3, P]
    cl = (r2 / SC_BF - ch.astype(np.float64)).astype(_bf16)
    fy = yf.astype(np.float64)
    fx = xf.astype(np.float64)
    fc = ch.astype(np.float64) + cl.astype(np.float64)
    dvec = -0.5 * (fy ** 2 + fx ** 2 + (fc ** 2).sum(0))  # [P]
    dh = dvec.astype(_bf16)
    dl = (dvec - dh.astype(np.float64)).astype(_bf16)
    ones = np.ones(P, _bf16)
    PL = np.stack([yf, xf, ch[0], ch[1], ch[2], ch[0], ch[1], ch[2],
                   cl[0], cl[1], cl[2], dh, dl, ones, ones], 0)
    QR = np.stack([yf, xf, ch[0], ch[1], ch[2], cl[0], cl[1], cl[2],
                   ch[0], ch[1], ch[2], ones, ones, dh, dl], 0)
    return np.ascontiguousarray(PL), np.ascontiguousarray(QR)


# ----------------------------------------------------------------------------
# device program
# ----------------------------------------------------------------------------

def _build_bass(niters=5):
    key = ("nc", niters)
    if key in _CACHE:
        return _CACHE[key]

    import concourse.bass as bass
    import concourse.bacc as bacc
    import concourse.tile as tile
    import concourse.mybir as mybir
    from concourse.masks import make_identity

    f32 = mybir.dt.float32
    bf16 = mybir.dt.bfloat16
    fp8 = mybir.dt.float8e4
    DR = mybir.MatmulPerfMode.DoubleRow
    AF = mybir.ActivationFunctionType
    ALU = mybir.AluOpType

    nc = bacc.Bacc("TRN2", num_devices=NCORES)

    unary_m = nc.dram_tensor("unary_m", [L, PSH], f32, kind="ExternalInput")
    plb = nc.dram_tensor("plb", [NF, P], bf16, kind="ExternalInput")
    qrb = nc.dram_tensor("qrb", [NF, PSH], bf16, kind="ExternalInput")
    bmat = nc.dram_tensor("bmat", [W, W], bf16, kind="ExternalInput")
    bmy = nc.dram_tensor("bmy", [H, NYS], bf16, kind="ExternalInput")
    selb = nc.dram_tensor("selb", [NI, NJ], bf16, kind="ExternalInput")
    qout = nc.dram_tensor("qout", [NJ, 128, L], f32, kind="ExternalOutput")

    rg = [list(range(NCORES))]
    COLS = ((0, 512), (512, 512), (1024, 128))
    # fp8 DoubleRow matvec: 2 contraction k-tiles per instruction, moving
    # free dim 2n <= 512 so column groups of <= 256; grouped to the same
    # 3 PSUM bank ranges as COLS for the bank-major epilogue pipeline.
    BANKCOLS = (((0, 256), (256, 256)), ((512, 256), (768, 256)),
                ((1024, 128),))
    HXL = XL // 2
    NIH = NI // 2          # 36: partials-AG half
    CLH = CLEN // 2

    with tile.TileContext(nc) as tc:
        with tc.tile_pool(name="dram", bufs=1, space="DRAM") as dram:
            qsh_d = dram.tile([NJ, 128, L], fp8)    # my q shard, AG input
            qfl_d = dram.tile([NCORES, QLEN], fp8)  # AG output
            pin_d = dram.tile([2, CLH], bf16)        # colsum partials in
            pout_d = dram.tile([2, NCORES, CLH], bf16)
            spd = dram.tile([NG, LG * PSH], bf16)    # spatial bounce

            with tc.tile_pool(name="persist", bufs=1) as persist:
                Mt = persist.tile([128, NI, PSH], fp8, name="Mt")
                U_sb = persist.tile([L, PSH], f32)
                U_pix = persist.tile([128, NJ, L], f32)
                ident = persist.tile([L, L], f32)
                make_identity(nc, ident[:])
                idnys = persist.tile([NYS, NYS], bf16)
                make_identity(nc, idnys[:])
                id84 = persist.tile([LG * NYS, LG * NYS], bf16)
                make_identity(nc, id84[:])
                id128 = persist.tile([128, 128], bf16)
                make_identity(nc, id128[:])
                QRs = persist.tile([NF, PSH], bf16)
                nc.sync.dma_start(QRs[:], qrb[:, :])
                Bm = persist.tile([W, W], bf16)
                nc.sync.dma_start(Bm[:], bmat[:, :])
                By = persist.tile([H, NYS], bf16)
                nc.sync.dma_start(By[:], bmy[:, :])
                csacc = persist.tile([128, NI], f32)
                rnX = persist.tile([128, NI, L], bf16)
                rnpix = persist.tile([128, NJ], f32)

                with (
                    tc.tile_pool(name="itq", bufs=1) as itq,
                    tc.tile_pool(name="ite", bufs=3) as ite,
                    tc.tile_pool(name="spt", bufs=1) as spt,
                    tc.tile_pool(name="ittp", bufs=1, space="PSUM") as ittp,
                ):
                    # ---- prologue: U, q0, early AllGather ---------------
                    ut = itq.tile([L, PSH], f32, tag="qh")
                    nc.sync.dma_start(ut[:], unary_m[:, :])
                    nc.vector.tensor_scalar(ut[:], ut[:], 1e-5, 1.0,
                                            op0=ALU.max, op1=ALU.min)
                    nc.scalar.activation(U_sb[:], ut[:], AF.Ln)
                    e9 = ite.tile([128, NJ, L], f32, tag="e9")
                    zz = ite.tile([128, NJ], f32, tag="zz")
                    rz = ite.tile([128, NJ], f32, tag="rz")
                    tpall = ittp.tile([128, NJ * L], f32, tag="tp")
                    for jc in range(NJ):
                        tp = tpall[:, jc * L:(jc + 1) * L]
                        nc.tensor.transpose(
                            tp, U_sb[:, jc * 128:(jc + 1) * 128], ident[:])
                        nc.scalar.activation(e9[:, jc, :], tp, AF.Exp,
                                             accum_out=zz[:, jc:jc + 1])
                    nc.vector.tensor_copy(
                        U_pix[:].rearrange("a b c -> a (b c)"), tpall[:])
                    nc.vector.reciprocal(rz[:], zz[:])
                    qm0 = ite.tile([128, NJ, L], fp8, tag="qm")
                    for jc in range(NJ):
                        nc.vector.tensor_scalar_mul(
                            qm0[:, jc, :], e9[:, jc, :], rz[:, jc:jc + 1])
                    nc.gpsimd.dma_start(
                        qsh_d.rearrange("a b c -> b a c"), qm0[:])
                    nc.gpsimd.collective_compute(
                        "AllGather", mybir.AluOpType.bypass,
                        replica_groups=rg,
                        ins=[qsh_d.rearrange("a b c -> (a b c)")],
                        outs=[qfl_d.rearrange("a b -> (a b)")])

                    # ---- E = exp(T') build; halfwise colsum partial AGs -
                    cstp_cm = tc.tile_pool(name="csp", bufs=1)
                    cstp = cstp_cm.__enter__()
                    with (
                        tc.tile_pool(name="slab", bufs=2) as slabp,
                        tc.tile_pool(name="eps", bufs=2, space="PSUM") as eps,
                    ):
                        for sb in range(NSLAB):
                            sl = slabp.tile([NF, PSH], bf16, tag="sl")
                            nc.sync.dma_start(
                                sl[:], plb[:, sb * PSH:(sb + 1) * PSH])
                            for k in range(NI // NSLAB):
                                ic = sb * (NI // NSLAB) + k
                                ps = eps.tile([128, PSH], f32, tag="eps")
                                lh = sl[:, k * 128:(k + 1) * 128]
                                for (o, n) in COLS:
                                    nc.tensor.matmul(
                                        ps[:, o:o + n], lh,
                                        QRs[:, o:o + n],
                                        start=True, stop=True)
                                nc.scalar.activation(
                                    Mt[:, ic, :], ps[:], AF.Exp,
                                    accum_out=csacc[:, ic:ic + 1])
                                if ic == NIH - 1 or ic == NI - 1:
                                    hf = 0 if ic < NIH else 1
                                    io = hf * NIH
                                    csb = cstp.tile([128, NIH], bf16,
                                                    tag=f"csb{hf}")
                                    nc.vector.tensor_copy(
                                        csb[:], csacc[:, io:io + NIH])
                                    nc.gpsimd.dma_start(
                                        pin_d[hf, :].rearrange(
                                            "(p i) -> p i", p=128, i=NIH),
                                        csb[:])
                                    nc.gpsimd.collective_compute(
                                        "AllGather", mybir.AluOpType.bypass,
                                        replica_groups=rg,
                                        ins=[pin_d[hf, :]],
                                        outs=[pout_d[hf].rearrange(
                                            "a b -> (a b)")])

                        # iter-1 gather DMAs first: only depend on the
                        # q0 AllGather, must not queue behind rn staging
                        selt = cstp.tile([NI, NJ], bf16)
                        nc.sync.dma_start(selt[:], selb[:, :])
                        Qi1 = itq.tile([H, XL], fp8, tag="Qi")
                        nc.sync.dma_start(
                            Qi1[:], qfl_d.rearrange(
                                "a b -> (a b)").rearrange(
                                "(y xl) -> y xl", y=H, xl=XL))
                        qf1 = itq.tile([128, NI, L], fp8, tag="qf")
                        qf18 = itq.tile([128, NI, LPAD], fp8, tag="qf8")
                        nc.vector.memset(qf18[:, :, L:LPAD], 0.0)
                        nc.sync.dma_start(
                            qf1[:, :, :],
                            qfl_d.rearrange("c q -> (c q)").rearrange(
                                "(a b c2) -> b a c2",
                                a=NI, b=128, c2=L))

                        def rn_half(hf, eng, fill_rnx=True):
                            io = hf * NIH
                            st = cstp.tile([128, NCORES, NIH], bf16,
                                           tag="st")
                            eng.dma_start(
                                st[:],
                                pout_d[hf].rearrange(
                                    "c (p i) -> p c i", p=128, i=NIH))
                            cs = cstp.tile([128, NIH], f32, tag=f"cs{hf}")
                            nc.vector.tensor_add(cs[:], st[:, 0, :],
                                                 st[:, 1, :])
                            for c in range(2, NCORES):
                                nc.vector.tensor_add(cs[:], cs[:],
                                                     st[:, c, :])
                            nc.vector.reciprocal(cs[:], cs[:])
                            nc.scalar.activation(cs[:], cs[:], AF.Sqrt)
                            if fill_rnx:
                                for l in range(L):
                                    nc.vector.tensor_copy(
                                        rnX[:, io:io + NIH, l], cs[:])
                            return cs

                        def scale_half(qf, hf):
                            for c in range(hf * 4, hf * 4 + 4):
                                cs_ = slice(c * NJ, (c + 1) * NJ)
                                nc.vector.tensor_mul(
                                    qf18[:, cs_, 0:L],
                                    qf[:, cs_, :],
                                    rnX[:, cs_, :])

                        csA = rn_half(0, nc.sync)
                        scale_half(qf1, 0)
                        csB = rn_half(1, nc.sync)
                        scale_half(qf1, 1)

                        # J-side rn via on-chip PE transpose of the rn
                        # vector (the transposed-layout DRAM gather costs
                        # 16us of 2-byte-run DMA scatter; this is ~6 ops).
                        # The 4x factor rides in selb (host fills 4.0).
                        rnSb = cstp.tile([128, NI], bf16)
                        nc.vector.tensor_copy(rnSb[:, 0:NIH], csA[:])
                        nc.vector.tensor_copy(rnSb[:, NIH:NI], csB[:])
                        rnTp = ittp.tile([NI, 128], bf16, tag="tp")
                        nc.tensor.transpose(rnTp[:], rnSb[:], id128[:])
                        rnT = cstp.tile([NI, 128], bf16)
                        nc.vector.tensor_copy(rnT[:], rnTp[:])

                        def rn_jside():
                            # one-hot extraction; inside iter-1 after its
                            # matvec so it does not block the PE queue
                            rnpp = ittp.tile([128, NJ], f32, tag="tp")
                            nc.tensor.matmul(rnpp[:], rnT[:], selt[:],
                                             start=True, stop=True)
                            nc.vector.tensor_copy(rnpix[:], rnpp[:])

                    # ---- iterations 1..niters --------------------------
                    with (
                        tc.tile_pool(name="itps", bufs=1,
                                     space="PSUM") as itps,
                        tc.tile_pool(name="spps", bufs=1,
                                     space="PSUM") as spps,
                    ):
                        for it in range(1, niters + 1):
                            if it == 1:
                                Qi = Qi1
                                qf8 = qf18
                            else:
                                Qi = itq.tile([H, XL], fp8, tag="Qi")
                                nc.sync.dma_start(
                                    Qi[:], qfl_d.rearrange(
                                        "a b -> (a b)").rearrange(
                                        "(y xl) -> y xl", y=H, xl=XL))
                                qf = itq.tile([128, NI, L], fp8, tag="qf")
                                qf8 = itq.tile([128, NI, LPAD], fp8,
                                                tag="qf8")
                                qflv = qfl_d.rearrange(
                                    "c q -> (c q)").rearrange(
                                    "(a b c2) -> b a c2",
                                    a=NI, b=128, c2=L)
                                for h in range(4):
                                    hs = slice(h * NI // 4,
                                               (h + 1) * NI // 4)
                                    nc.sync.dma_start(
                                        qf[:, hs, :], qflv[:, hs, :])
                                    nc.vector.tensor_mul(
                                        qf8[:, hs, 0:L],
                                        qf[:, hs, :],
                                        rnX[:, hs, :])

                            # matvec with spatial ops interleaved in the
                            # PE stream; for it>=2 run bank-major so each
                            # PSUM bank's columns finalize early and the
                            # softmax overlaps the remaining banks
                            ps = itps.tile([LPAD, PSH], f32, tag="qbps")
                            zs = spt.tile([NYS, W, L], bf16, tag="zs")
                            spx = spt.tile([128, NJ, L], bf16, tag="spx")
                            usp = spt.tile([128, NJ, L], f32, tag="usp")
                            qs = itq.tile([L, PSH], f32, tag="qh")
                            zt = spt.tile([W, NYS * L], bf16, tag="zt2")
                            sp3 = spt.tile([NYS * LG, NG, W], bf16,
                                           tag="sp3b")
                            spT = spt.tile([W, NG, NYS, LG], bf16,
                                           tag="spT")
                            e9 = ite.tile([128, NJ, L], f32, tag="e9")
                            zz = ite.tile([128, NJ], f32, tag="zz")
                            rz = ite.tile([128, NJ], f32, tag="rz")
                            m1 = ite.tile([128, NJ, L], f32, tag="m1")
                            tpall = ittp.tile([128, NJ * L], f32, tag="tp")

                            SL1, SL2, SL2B = (60, 96, 114)

                            def spatial_slot(ic):
                                if ic == 0:
                                    for h in range(2):
                                        zmp = spps.tile([NYS, HXL], f32,
                                                        tag="zm")
                                        b0 = h * HXL
                                        for (o, n) in ((0, 512),
                                                       (512, HXL - 512)):
                                            nc.tensor.matmul(
                                                zmp[:, o:o + n], By[:],
                                                Qi[:, b0 + o:b0 + o + n],
                                                start=True, stop=True,
                                                skip_group_check=True)
                                        nc.vector.tensor_copy(
                                            zs[:, h * (W // 2):
                                               (h + 1) * (W // 2), :]
                                            .rearrange("a b c -> a (b c)"),
                                            zmp[:])
                                if ic == SL1:
                                    ztp = spps.tile([W, NYS * L], bf16,
                                                    tag="zt")
                                    for l in range(L):
                                        nc.tensor.transpose(
                                            ztp[:, l * NYS:(l + 1) * NYS],
                                            zs[:, :, l], idnys[:])
                                    nc.vector.tensor_copy(zt[:], ztp[:])
                                if ic == SL2:
                                    sp3p = spps.tile([NYS * LG, W], f32,
                                                     tag="sp3")
                                    for g in range(NG):
                                        nc.tensor.matmul(
                                            sp3p[:],
                                            zt[:, g * NYS * LG:
                                               (g + 1) * NYS * LG],
                                            Bm[:], start=True, stop=True,
                                            skip_group_check=True)
                                        nc.vector.tensor_copy(sp3[:, g, :],
                                                              sp3p[:])
                                if ic == SL2B:
                                    # transpose sp3 on-chip so the DRAM
                                    # bounce lands pixel-major: write runs
                                    # of LG=7 els, read one contiguous DMA
                                    spdf = spd.rearrange("a b -> (a b)")
                                    spdp = spdf.rearrange(
                                        "(y x l) -> x y l",
                                        y=NYS, x=W, l=L)
                                    for g in range(NG):
                                        spTp = spps.tile([W, LG * NYS],
                                                         bf16, tag="zm")
                                        nc.tensor.transpose(
                                            spTp[:], sp3[:, g, :], id84[:])
                                        nc.vector.tensor_copy(
                                            spT[:, g, :, :],
                                            spTp[:].rearrange(
                                                "x (l y) -> x y l",
                                                l=LG, y=NYS))
                                        nc.sync.dma_start(
                                            spdp[:, :, g * LG:(g + 1) * LG],
                                            spT[:, g, :, :])
                                    nc.sync.dma_start(
                                        spx[:],
                                        spdf.rearrange(
                                            "(a b c) -> b a c",
                                            a=NJ, b=128, c=L))
                                    nc.vector.tensor_add(
                                        usp[:].rearrange("a b c -> a (b c)"),
                                        U_pix[:].rearrange(
                                            "a b c -> a (b c)"),
                                        spx[:].rearrange(
                                            "a b c -> a (b c)"))
                                return

                            def softmax_jc(jc):
                                tp = tpall[:, jc * L:(jc + 1) * L]
                                nc.tensor.transpose(
                                    tp, qs[:, jc * 128:(jc + 1) * 128],
                                    ident[:])
                                nc.vector.scalar_tensor_tensor(
                                    m1[:, jc, :], tp, rnpix[:, jc:jc + 1],
                                    usp[:, jc, :],
                                    op0=ALU.mult, op1=ALU.add)
                                nc.scalar.activation(
                                    e9[:, jc, :], m1[:, jc, :], AF.Exp,
                                    accum_out=zz[:, jc:jc + 1])

                            if it == 1:
                                # chunk-major: B-half chunks arrive late
                                for ic2 in range(0, NI, 2):
                                    spatial_slot(ic2 * 3)
                                    for bcols in BANKCOLS:
                                        for (o, n) in bcols:
                                            nc.tensor.matmul(
                                                ps[:, o:o + n],
                                                qf8[:, ic2:ic2 + 2, :],
                                                Mt[:, ic2:ic2 + 2, o:o + n],
                                                start=(ic2 == 0),
                                                stop=(ic2 == NI - 2),
                                                perf_mode=DR,
                                                skip_group_check=True)
                                rn_jside()
                                nc.vector.tensor_copy(qs[:], ps[0:L, :])
                                for jc in range(NJ):
                                    softmax_jc(jc)
                            else:
                                BANKJC = ((0, 4), (4, 8), (8, 9))
                                last = (it == niters)
                                qm = ite.tile([128, NJ, L],
                                              f32 if last else fp8,
                                              tag="qm")

                                def epi_tail(bi):
                                    j0, j1 = BANKJC[bi]
                                    nc.vector.reciprocal(rz[:, j0:j1],
                                                         zz[:, j0:j1])
                                    for jc in range(j0, j1):
                                        nc.vector.tensor_scalar_mul(
                                            qm[:, jc, :], e9[:, jc, :],
                                            rz[:, jc:jc + 1])
                                    if not last:
                                        nc.sync.dma_start(
                                            qsh_d[j0:j1].rearrange(
                                                "a b c -> b a c"),
                                            qm[:, j0:j1, :])

                                def epi_bank(bi):
                                    j0, j1 = BANKJC[bi]
                                    for jc in range(j0, j1):
                                        softmax_jc(jc)
                                    epi_tail(bi)

                                for bi, (o, n) in enumerate(COLS):
                                    for ic2 in range(0, NI, 2):
                                        spatial_slot(bi * 72 + ic2)
                                        for (o2, n2) in BANKCOLS[bi]:
                                            nc.tensor.matmul(
                                                ps[:, o2:o2 + n2],
                                                qf8[:, ic2:ic2 + 2, :],
                                                Mt[:, ic2:ic2 + 2,
                                                   o2:o2 + n2],
                                                start=(ic2 == 0),
                                                stop=(ic2 == NI - 2),
                                                perf_mode=DR,
                                                skip_group_check=True)
                                    nc.vector.tensor_copy(
                                        qs[:, o:o + n], ps[0:L, o:o + n])
                                    if bi > 0:
                                        epi_bank(bi - 1)
                                epi_bank(2)
                                if not last:
                                    nc.gpsimd.collective_compute(
                                        "AllGather", mybir.AluOpType.bypass,
                                        replica_groups=rg,
                                        ins=[qsh_d.rearrange(
                                            "a b c -> (a b c)")],
                                        outs=[qfl_d.rearrange(
                                            "a b -> (a b)")])
                                else:
                                    nc.sync.dma_start(
                                        qout[:, :, :].rearrange(
                                            "a b c -> b a c"), qm[:])
                                continue
                            nc.vector.reciprocal(rz[:], zz[:])
                            qm = ite.tile([128, NJ, L],
                                          f32 if it == niters else fp8,
                                          tag="qm")
                            for jc in range(NJ):
                                nc.vector.tensor_scalar_mul(
                                    qm[:, jc, :], e9[:, jc, :],
                                    rz[:, jc:jc + 1])
                            if it < niters:
                                nc.sync.dma_start(
                                    qsh_d.rearrange("a b c -> b a c"), qm[:])
                                nc.gpsimd.collective_compute(
                                    "AllGather", mybir.AluOpType.bypass,
                                    replica_groups=rg,
                                    ins=[qsh_d.rearrange(
                                        "a b c -> (a b c)")],
                                    outs=[qfl_d.rearrange("a b -> (a b)")])
                            else:
                                nc.sync.dma_start(
                                    qout[:, :, :].rearrange("a b c -> b a c"),
                                    qm[:])
                        cstp_cm.__exit__(None, None, None)

    nc.finalize()
    _CACHE[key] = nc
    return nc


# ----------------------------------------------------------------------------
# host entry point
# ----------------------------------------------------------------------------

def _in_maps(unary, ref):
    B, _, _ = _host_constants()
    PL, QR = _feature_rows(ref)
    u2 = np.ascontiguousarray(np.asarray(unary, np.float32).reshape(L, P))
    bmat = np.ascontiguousarray(B.astype(_bf16))
    maps = []
    for c in range(NCORES):
        sl = slice(c * PSH, (c + 1) * PSH)
        sel = np.zeros((NI, NJ), _bf16)
        for jc in range(NJ):
            sel[c * NJ + jc, jc] = 4.0   # folds COMPAT_BF into rn_j
        bmyc = np.ascontiguousarray(
            (2.0 * B[:, c * NYS:(c + 1) * NYS]).astype(_bf16))
        maps.append({
            "unary_m": np.ascontiguousarray(u2[:, sl]),
            "plb": PL,
            "qrb": np.ascontiguousarray(QR[:, sl]),
            "bmat": bmat,
            "bmy": bmyc,
            "selb": sel,
        })
    return maps


def kernel(unary: np.ndarray, ref: np.ndarray) -> np.ndarray:
    from concourse import bass_utils

    nc = _build_bass()
    in_maps = _in_maps(unary, ref)

    global LAST_RESULT
    res = bass_utils.run_bass_kernel_spmd(nc, in_maps,
                                          core_ids=list(range(NCORES)),
                                          trace=TRACE)
    LAST_RESULT = res
    shards = [res.results[c]["qout"].reshape(PSH, L) for c in range(NCORES)]
    qfull = np.concatenate(shards, 0)          # [P, L]
    out = qfull.T.reshape(1, L, H, W).astype(np.float32)
    return out


if __name__ == "__main__":
    u = np.random.rand(1, L, H, W).astype(np.float32)
    r = (np.random.rand(1, 3, H, W) * 255).astype(np.float32)
    o = kernel(u, r)
    print(o.shape, o.dtype, o.sum())

